# revision 27
# baseline (speedup 1.0000x reference)
"""Causal multi-head attention (B=1, S=2048, H=16, D=128, fp32) on 8 TRN2
NeuronCores — head parallelism (2 heads/core), no collectives.

Per-core engine model (from trace analysis):
  ACT exp stream: 40 instrs, (free+~190cyc)/1.2GHz  -> ~37us, the binding engine
  PE  mm1+mm2+diag-l+contract ~35us
  DVE ops: fresh-output fp16 SBUF adds ~417ns; in-place or PSUM-input ops
  ~690-1100ns -> keep everything fresh-output fp16, minimize op count
  GpSimd: [P,128] mask-mults ~453ns; [P,512] adds ~1258ns (too slow - masks only)

Layout: Q/K pre-transposed on host to [d, h, s] fp16 (contraction on
partitions), V natural [s, h, d] fp16, out [h, d, s] fp16 (host upcasts).
fp16 streams the PE at 1 col/cycle (warm 2.4GHz) like bf16.

Structure (vs the 65.4us baseline):
  - flat software pipeline ACROSS s-blocks, block order (1,2,3,0)
  - denominator: per full group pair = t0+t1 (DVE fresh), acc' = acc + pair
    (DVE fresh) emitted at group time -> reduction is DONE when the block
    ends; diag contributions stay on PE (ones-matmuls into psum_l)
  - psum_l is ONE aliased bank (h0 then h1): h1's entire block-end chain
    (diag-l matmuls, contract, recip, mul, dma) is DEFERRED one group so it
    never head-of-line-blocks the PE queue behind the next block's mm1s
  - last block (b=0): h1's l goes to a spare ps-pool tile instead so both
    heads' normalize chains run in parallel in the tail
  - input DMA: first-gating chunks (k0, q1) split per-head/halved on sync,
    V chunks on gpsimd, later chunks combined [d, 2h, 512]
  - output fp16, one DMA per (head, block)
"""

import math

import numpy as np

import concourse.mybir as mybir
import concourse.tile as tile
from concourse import bacc
from concourse.masks import make_upper_triangular

S = 2048
H = 16
D = 128
HC = 2  # heads per core
NCORES = 8
P = 128
SBLK = 512  # s-block width
NT = S // P  # 16 t tiles
NB = S // SBLK  # 4 s blocks / chunks
TPB = SBLK // P  # 4 t tiles per s block
SCALE = 1.0 / math.sqrt(D)

F32 = mybir.dt.float32
FP16 = mybir.dt.float16

BLOCK_ORDER = (1, 2, 3, 0)
# warmup must span the full ~3.4us HAM activity window BY ITSELF: the early
# blocks are DMA-trickled and can't fill it, leaving the PE at half clock
# into block 2 (measured: 18 warmups -> warm at ~18-22us; 40 -> ~12us)
N_WARMUP = 32


def _groups_of(b):
    """Groups of two t-tiles sharing one 2-bank psum + one exp:
    (i0, i1, s_lo0, s_lo1, is_diag)."""
    n_full = TPB * b
    gs = [(ip, ip + 1, 0, 0, False) for ip in range(0, n_full, 2)]
    gs += [
        (n_full, n_full + 1, 0, P, True),
        (n_full + 2, n_full + 3, 2 * P, 3 * P, True),
    ]
    return gs


def build_nc():
    nc = bacc.Bacc("TRN2", target_bir_lowering=False, debug=False, num_devices=NCORES)
    qt_d = nc.dram_tensor("qt", [HC, D, S], FP16, kind="ExternalInput").ap()
    kt_d = nc.dram_tensor("kt", [HC, D, S], FP16, kind="ExternalInput").ap()
    v_d = nc.dram_tensor("v", [S, HC, D], FP16, kind="ExternalInput").ap()
    ot_d = nc.dram_tensor("ot", [HC, D, S], FP16, kind="ExternalOutput").ap()

    with tile.TileContext(nc) as tc:
        with (
            tc.tile_pool(name="consts", bufs=1) as cpool,
            tc.tile_pool(name="big", bufs=1) as bigpool,
            tc.tile_pool(name="exp", bufs=12) as epool,
            tc.tile_pool(name="norm", bufs=8) as npool,
            tc.tile_pool(name="psum_s", bufs=2, space="PSUM") as ps_pool,
            tc.tile_pool(name="psum_o", bufs=3, space="PSUM") as po_pool,
            tc.tile_pool(name="psum_l", bufs=1, space="PSUM") as pl_pool,
        ):
            ones = cpool.tile([P, P], FP16, tag="ones")
            nc.vector.memset(ones, 1.0)
            warm_ps = pl_pool.tile([P, SBLK], F32, tag="pl", name="warm_ps")
            for w in range(N_WARMUP):
                nc.tensor.matmul(
                    warm_ps[:, :P],
                    ones[:],
                    ones[:],
                    start=True,
                    stop=True,
                    skip_group_check=True,
                )
            tri = cpool.tile([P, P], FP16, tag="tri")
            make_upper_triangular(nc, tri, val=1.0, diag=True)

            ktre = kt_d.rearrange("h d s -> d h s")
            qtre = qt_d.rearrange("h d s -> d h s")
            vre = v_d.rearrange("(i p) h d -> p i h d", p=P)
            kt_c = {}
            qt_c = {}
            vb_c = {}
            for c in range(NB):
                kt_c[c] = bigpool.tile([P, HC, SBLK], FP16, tag=f"ktc{c}", name=f"ktc{c}")
                qt_c[c] = bigpool.tile([P, HC, SBLK], FP16, tag=f"qtc{c}", name=f"qtc{c}")
                vb_c[c] = bigpool.tile([P, TPB, HC, D], FP16, tag=f"vbc{c}", name=f"vbc{c}")

            def cs(c):
                return slice(c * SBLK, (c + 1) * SBLK)

            # DMA facts (measured): ~700ns issue per instruction, ~1.5-2.2us
            # ring cold-start before the first packet, then ~100GB/s per
            # issuing ring (sync/scalar/gpsimd are separate rings, served
            # in order per ring).  K stream on sync, Q stream on scalar,
            # V on gpsimd; a tiny primer DMA per ring eats the cold-start.
            # rings stream ~100GB/s each with ~700ns issue + ~1.5us wake
            # (primer).  The scalar queue carries ONLY qt1 — any more DMA
            # issues there sit in front of the first ACTIVATE (in-order
            # queue) and stall the whole exp stream.
            scratch = cpool.tile([P, 2], FP16, tag="dma_primer")
            nc.sync.dma_start(scratch[:], ktre[:, 0, 0:2])
            nc.sync.dma_start(kt_c[0][:, 0, :], ktre[:, 0, cs(0)])
            nc.scalar.dma_start(qt_c[1][:, 0, :], qtre[:, 0, cs(1)])
            nc.sync.dma_start(kt_c[1][:, 0, :], ktre[:, 0, cs(1)])
            nc.scalar.dma_start(qt_c[1][:, 1, :], qtre[:, 1, cs(1)])
            nc.gpsimd.dma_start(vb_c[0][:], vre[:, 0:TPB])
            nc.sync.dma_start(kt_c[0][:, 1, :], ktre[:, 1, cs(0)])
            nc.sync.dma_start(kt_c[1][:, 1, :], ktre[:, 1, cs(1)])
            nc.gpsimd.dma_start(vb_c[1][:], vre[:, TPB : 2 * TPB])
            nc.sync.dma_start(qt_c[2][:], qtre[:, :, cs(2)])
            nc.sync.dma_start(kt_c[2][:], ktre[:, :, cs(2)])
            nc.gpsimd.dma_start(vb_c[2][:], vre[:, 2 * TPB : 3 * TPB])
            nc.sync.dma_start(qt_c[3][:], qtre[:, :, cs(3)])
            nc.sync.dma_start(kt_c[3][:], ktre[:, :, cs(3)])
            nc.gpsimd.dma_start(vb_c[3][:], vre[:, 3 * TPB : 4 * TPB])
            nc.sync.dma_start(qt_c[0][:], qtre[:, :, cs(0)])

            def kt_tile(h, i):
                return kt_c[i // TPB][:, h, (i % TPB) * P : (i % TPB + 1) * P]

            def v_tile(h, i):
                return vb_c[i // TPB][:, i % TPB, h, :]

            psum_o = {}
            psum_l = {}
            acc = {}  # (h, b) -> current fresh fp16 accumulator tile
            expt_of = {}
            l_alt = {}  # b0 h1's l target (ps-pool tile)

            def ensure_block(b):
                if (0, b) in psum_o:
                    return
                for h in range(HC):
                    psum_o[h, b] = po_pool.tile(
                        [P, SBLK], F32, tag="po", name=f"po{h}_{b}"
                    )
                    if b:  # b0's l lives in a spare ps tile instead
                        psum_l[h, b] = pl_pool.tile(
                            [P, SBLK], F32, tag="pl", name=f"pl{h}_{b}"
                        )

            def emit_mm1(h, b, grp):
                i0, i1, s0, s1, is_diag = grp
                psum_s = ps_pool.tile([P, 2, SBLK], F32, tag="ps", name=f"ps{h}_{b}_{i0}")
                expt = epool.tile([P, 2, SBLK], FP16, tag="expt", name=f"ex{h}_{b}_{i0}")
                for j, (i, s_lo) in enumerate(((i0, s0), (i1, s1))):
                    nc.tensor.matmul(
                        psum_s[:, j, s_lo:],
                        kt_tile(h, i),
                        qt_c[b][:, h, s_lo:],
                        start=True,
                        stop=True,
                    )
                nc.scalar.activation(
                    expt[:, :, s0:],
                    psum_s[:, :, s0:],
                    mybir.ActivationFunctionType.Exp,
                    scale=SCALE,
                )
                if is_diag:
                    # h1's masks of the last block's LAST group go to DVE so
                    # the tail's final four masks run on two engines at once
                    meng = nc.vector if (b == 0 and h == 1 and s0 != 0) else nc.gpsimd
                    for j, s_lo in enumerate((s0, s1)):
                        meng.tensor_mul(
                            out=expt[:, j, s_lo : s_lo + P],
                            in0=expt[:, j, s_lo : s_lo + P],
                            in1=tri[:],
                        )
                expt_of[h, b, i0] = expt

            def emit_diag_l(h, b, grp, target):
                """ones-matmul denominator contributions of a diag group."""
                i0, i1, s0, s1, _ = grp
                n_full = TPB * b
                last_i = n_full + TPB - 1
                expt = expt_of[h, b, i0]
                for j, (i, s_lo) in enumerate(((i0, s0), (i1, s1))):
                    nc.tensor.matmul(
                        target[:, s_lo:],
                        ones[:],
                        expt[:, j, s_lo:],
                        start=(i == n_full),
                        stop=(i == last_i and n_full == 0),
                        skip_group_check=True,
                    )

            def emit_mm2(h, b, grp, with_diag_l=True):
                i0, i1, s0, s1, is_diag = grp
                n_full = TPB * b
                last_i = n_full + TPB - 1
                expt = expt_of[h, b, i0]
                for j, (i, s_lo) in enumerate(((i0, s0), (i1, s1))):
                    nc.tensor.matmul(
                        psum_o[h, b][:, s_lo:],
                        v_tile(h, i),
                        expt[:, j, s_lo:],
                        start=(i == 0),
                        stop=(i == last_i),
                        skip_group_check=True,
                    )
                if is_diag:
                    if with_diag_l:
                        emit_diag_l(h, b, grp, psum_l[h, b])
                        expt_of.pop((h, b, i0))
                    # else: deferred — tile stays alive in expt_of
                else:
                    expt_of.pop((h, b, i0))
                    # fresh-output fp16 adds on DVE (in-place accumulate and
                    # gpsimd [P,512] adds are 2.5-3x slower)
                    pair = npool.tile([P, SBLK], FP16, tag="pair", name=f"pr{h}_{b}_{i0}")
                    nc.vector.tensor_add(
                        out=pair[:], in0=expt[:, 0, :], in1=expt[:, 1, :]
                    )
                    if (h, b) not in acc:
                        acc[h, b] = pair
                    else:
                        nacc = npool.tile(
                            [P, SBLK], FP16, tag="acc", name=f"ac{h}_{b}_{i0}"
                        )
                        nc.vector.tensor_add(
                            out=nacc[:], in0=acc[h, b][:], in1=pair[:]
                        )
                        acc[h, b] = nacc

            def emit_end(h, b, ltile):
                bs = slice(b * SBLK, (b + 1) * SBLK)
                if b:
                    # contract the DVE pair-sums over the partition dim
                    nc.tensor.matmul(
                        ltile[:],
                        ones[:],
                        acc[h, b][:],
                        start=False,
                        stop=True,
                        skip_group_check=True,
                    )
                recip = npool.tile([P, SBLK], F32, tag="recip", name=f"rc{h}_{b}")
                nc.vector.reciprocal_approx_fast(out=recip[:], in_=ltile[:])
                otn = npool.tile([P, SBLK], FP16, tag="otn", name=f"ot{h}_{b}")
                nc.vector.tensor_mul(out=otn[:], in0=psum_o[h, b][:], in1=recip[:])
                nc.sync.dma_start(ot_d[h, :, bs], otn[:])

            def emit_half_end_both(b, lps, half):
                """Normalize+store one query-half of the last block (both
                heads) as soon as causality makes it final: ONE joint recip
                over [P, 2, 256], then per-head mul + store."""
                hw2 = SBLK // 2
                hs = slice(half * hw2, (half + 1) * hw2)
                ds = slice(b * SBLK + half * hw2, b * SBLK + (half + 1) * hw2)
                recip = npool.tile([P, 2, hw2], F32, tag="reciph", name=f"rch_{b}_{half}")
                nc.vector.reciprocal_approx_fast(out=recip[:], in_=lps[:, :, hs])
                for h in range(HC):
                    otn = npool.tile([P, hw2], FP16, tag="otnh", name=f"oth{h}_{b}_{half}")
                    nc.vector.tensor_mul(
                        out=otn[:], in0=psum_o[h, b][:, hs], in1=recip[:, 1 - h, :]
                    )
                    (nc.sync if h == 0 else nc.gpsimd).dma_start(ot_d[h, :, ds], otn[:])

            # ---- flat pipeline across blocks at (head, group) granularity,
            # lookahead 2 jobs (= one group) ----
            jobs = []
            for b in BLOCK_ORDER:
                gs = _groups_of(b)
                for gi, grp in enumerate(gs):
                    for h in range(HC):
                        jobs.append((h, b, grp, gi == len(gs) - 1))
            # h0-major start: h1's gating DMAs land ~1.2us after h0's, so
            # run h0's first two groups before touching h1
            jobs[0:4] = [jobs[0], jobs[2], jobs[1], jobs[3]]

            def emit_deferred_h1(db):
                for g2 in _groups_of(db):
                    if g2[4]:
                        emit_diag_l(1, db, g2, psum_l[1, db])
                        expt_of.pop((1, db, g2[0]))
                emit_end(1, db, psum_l[1, db])

            l_ps = None  # b0's l lives in a spare ps tile (both heads)

            def consume_job(job):
                nonlocal l_ps, deferred
                h, b, grp, lastg = job
                if b != 0:
                    emit_mm2(h, b, grp, with_diag_l=(h == 0))
                    if lastg:
                        if h == 0:
                            emit_end(0, b, psum_l[0, b])
                        else:
                            # h1 (psum_l alias): defer past the next mm1s
                            deferred = b
                    return
                # b0 (the last block, diag-only): per-group diag-l into the
                # spare ps tile; queries [0:256] are final after g0 (causal),
                # so normalize+store that half while g1 still computes
                if l_ps is None:
                    l_ps = ps_pool.tile([P, 2, SBLK], F32, tag="ps", name="l_b0")
                half = 0 if grp[0] == 0 else 1
                emit_mm2(h, b, grp, with_diag_l=False)
                emit_diag_l(h, b, grp, l_ps[:, 1 - h, :])
                expt_of.pop((h, b, grp[0]))
                if h == 1:  # both heads' diag-l for this half are in
                    emit_half_end_both(b, l_ps, half)

            LOOK = 2
            deferred = None  # block whose h1 end-chain is due
            defer_age = 0
            for k, job in enumerate(jobs):
                h, b, grp, lastg = job
                ensure_block(b)
                emit_mm1(h, b, grp)
                if deferred is not None:
                    defer_age += 1
                    if defer_age >= 2:
                        db = deferred
                        deferred = None
                        defer_age = 0
                        emit_deferred_h1(db)
                if k >= LOOK:
                    consume_job(jobs[k - LOOK])
            if deferred is not None:
                emit_deferred_h1(deferred)
                deferred = None
            for k in range(len(jobs) - LOOK, len(jobs)):
                consume_job(jobs[k])
    nc.compile()
    return nc


_NC_CACHE = None


def _get_nc():
    global _NC_CACHE
    if _NC_CACHE is None:
        _NC_CACHE = build_nc()
    return _NC_CACHE


def make_in_maps(query, key, value):
    query = np.asarray(query)
    key = np.asarray(key)
    value = np.asarray(value)
    in_maps = []
    for c in range(NCORES):
        hs = slice(c * HC, (c + 1) * HC)
        in_maps.append(
            {
                "qt": np.ascontiguousarray(
                    query[0, :, hs, :].transpose(1, 2, 0)
                ).astype(np.float16),
                "kt": np.ascontiguousarray(
                    key[0, :, hs, :].transpose(1, 2, 0)
                ).astype(np.float16),
                "v": np.ascontiguousarray(value[0, :, hs, :]).astype(np.float16),
            }
        )
    return in_maps


def kernel(query, key, value):
    from concourse.bass_utils import run_bass_kernel_spmd

    nc = _get_nc()
    in_maps = make_in_maps(query, key, value)
    res = run_bass_kernel_spmd(nc, in_maps, core_ids=list(range(NCORES)))
    out = np.empty((1, S, H, D), dtype=np.float32)
    for c in range(NCORES):
        # ot is [HC, D, S] fp16 -> [S, HC, D] fp32
        out[0, :, c * HC : (c + 1) * HC, :] = (
            res.results[c]["ot"].astype(np.float32).transpose(2, 0, 1)
        )
    return out


# revision 29
# speedup vs baseline: 1.0388x; 1.0388x over previous
"""Causal multi-head attention (B=1, S=2048, H=16, D=128, fp32) on 8 TRN2
NeuronCores — head parallelism (2 heads/core), no collectives — ~62.8us HW
exec (baseline 65.4us), rel err ~4.3e-4 vs the fp32 reference.

Per-core engine model (measured from ntff traces):
  ACT exp stream: 40 instrs x ~1000ns issue-to-issue = ~38.5us — the
    binding engine (36864 exp columns / 1.2GHz + ~170cyc/instr overhead;
    wider instrs are impossible: all 8 PSUM banks are committed).
  PE mm1+mm2+diag-l+contract ~35us warm.  HAM clock-gate: the PE runs at
    HALF clock until ~3.4us of sustained busy — the 32 warmup matmuls
    bridge the DMA lead-in so the whole stream runs at 2.4GHz.
  DVE: fresh-output fp16 SBUF adds ~417ns; in-place accumulates and
    PSUM-input ops ~690-1100ns -> denominator uses fresh-output chains.
  GpSimd: [P,128] mask-mults ~453ns; CANNOT touch PSUM; [P,512] adds
    ~1258ns (never used for adds).
  DMA: only sync/scalar/gpsimd queues can issue; ~700ns per issue,
    ~1.5-2.2us ring wake (absorbed by a primer), ~100GB/s per ring after.

Layout: Q/K pre-transposed on host to [d, h, s] fp16 (contraction on
partitions), V natural [s, h, d] fp16, out [h, d, s] fp16 (host upcasts).
fp16 streams the PE at 1 col/cycle (warm 2.4GHz) like bf16.

Structure (vs the 65.4us baseline):
  - flat software pipeline across s-blocks at (head, group) granularity
    with one-group lookahead; block order (1,2,3,0) ends on the smallest
    block; h0-major start hides h1's later-arriving first chunks
  - K chunks stream on the sync ring behind a primer DMA; q1 on the scalar
    ring (and NOTHING else there — more DMA issues would sit in front of
    the first ACTIVATE in scalar's in-order queue); V on the gpsimd ring
  - denominator: per full group pair = t0+t1 (DVE fresh), acc' = acc + pair
    (DVE fresh) emitted at group time -> reduction is DONE when the block
    ends; diag contributions stay on PE (ones-matmuls into psum_l)
  - psum_l is ONE aliased bank (h0 then h1): h1's entire block-end chain
    (diag-l matmuls, contract, recip, mul, dma) is DEFERRED one group so it
    never head-of-line-blocks the PE queue behind the next block's mm1s
  - last block (b=0): both heads' l goes to a spare ps-pool tile, and the
    normalize+store runs per query-HALF as soon as causality makes that
    half final (one joint [P,2,256] reciprocal for both heads); stores
    split h0->sync / h1->gpsimd
"""

import math

import numpy as np

import concourse.mybir as mybir
import concourse.tile as tile
from concourse import bacc
from concourse.masks import make_upper_triangular

S = 2048
H = 16
D = 128
HC = 2  # heads per core
NCORES = 8
P = 128
SBLK = 512  # s-block width
NT = S // P  # 16 t tiles
NB = S // SBLK  # 4 s blocks / chunks
TPB = SBLK // P  # 4 t tiles per s block
SCALE = 1.0 / math.sqrt(D)

F32 = mybir.dt.float32
FP16 = mybir.dt.float16

BLOCK_ORDER = (1, 2, 3, 0)
# warmup must span the full ~3.4us HAM activity window BY ITSELF: the early
# blocks are DMA-trickled and can't fill it, leaving the PE at half clock
# into block 2 (measured: 18 warmups -> warm at ~18-22us; 40 -> ~12us)
N_WARMUP = 32


def _groups_of(b):
    """Groups of two t-tiles sharing one 2-bank psum + one exp:
    (i0, i1, s_lo0, s_lo1, is_diag)."""
    n_full = TPB * b
    gs = [(ip, ip + 1, 0, 0, False) for ip in range(0, n_full, 2)]
    gs += [
        (n_full, n_full + 1, 0, P, True),
        (n_full + 2, n_full + 3, 2 * P, 3 * P, True),
    ]
    return gs


def build_nc():
    nc = bacc.Bacc("TRN2", target_bir_lowering=False, debug=False, num_devices=NCORES)
    qt_d = nc.dram_tensor("qt", [HC, D, S], FP16, kind="ExternalInput").ap()
    kt_d = nc.dram_tensor("kt", [HC, D, S], FP16, kind="ExternalInput").ap()
    v_d = nc.dram_tensor("v", [S, HC, D], FP16, kind="ExternalInput").ap()
    ot_d = nc.dram_tensor("ot", [HC, D, S], FP16, kind="ExternalOutput").ap()

    with tile.TileContext(nc) as tc:
        with (
            tc.tile_pool(name="consts", bufs=1) as cpool,
            tc.tile_pool(name="big", bufs=1) as bigpool,
            tc.tile_pool(name="exp", bufs=12) as epool,
            tc.tile_pool(name="norm", bufs=8) as npool,
            tc.tile_pool(name="psum_s", bufs=2, space="PSUM") as ps_pool,
            tc.tile_pool(name="psum_o", bufs=3, space="PSUM") as po_pool,
            tc.tile_pool(name="psum_l", bufs=1, space="PSUM") as pl_pool,
        ):
            ones = cpool.tile([P, P], FP16, tag="ones")
            nc.vector.memset(ones, 1.0)
            warm_ps = pl_pool.tile([P, SBLK], F32, tag="pl", name="warm_ps")
            for w in range(N_WARMUP):
                nc.tensor.matmul(
                    warm_ps[:, :P],
                    ones[:],
                    ones[:],
                    start=True,
                    stop=True,
                    skip_group_check=True,
                )
            tri = cpool.tile([P, P], FP16, tag="tri")
            make_upper_triangular(nc, tri, val=1.0, diag=True)

            ktre = kt_d.rearrange("h d s -> d h s")
            qtre = qt_d.rearrange("h d s -> d h s")
            vre = v_d.rearrange("(i p) h d -> p i h d", p=P)
            kt_c = {}
            qt_c = {}
            vb_c = {}
            for c in range(NB):
                kt_c[c] = bigpool.tile([P, HC, SBLK], FP16, tag=f"ktc{c}", name=f"ktc{c}")
                qt_c[c] = bigpool.tile([P, HC, SBLK], FP16, tag=f"qtc{c}", name=f"qtc{c}")
                vb_c[c] = bigpool.tile([P, TPB, HC, D], FP16, tag=f"vbc{c}", name=f"vbc{c}")

            def cs(c):
                return slice(c * SBLK, (c + 1) * SBLK)

            # DMA facts (measured): ~700ns issue per instruction, ~1.5-2.2us
            # ring cold-start before the first packet, then ~100GB/s per
            # issuing ring (sync/scalar/gpsimd are separate rings, served
            # in order per ring).  K stream on sync, Q stream on scalar,
            # V on gpsimd; a tiny primer DMA per ring eats the cold-start.
            # rings stream ~100GB/s each with ~700ns issue + ~1.5us wake
            # (primer).  The scalar queue carries ONLY qt1 — any more DMA
            # issues there sit in front of the first ACTIVATE (in-order
            # queue) and stall the whole exp stream.
            scratch = cpool.tile([P, 2], FP16, tag="dma_primer")
            nc.sync.dma_start(scratch[:], ktre[:, 0, 0:2])
            nc.sync.dma_start(kt_c[0][:, 0, :], ktre[:, 0, cs(0)])
            nc.scalar.dma_start(qt_c[1][:, 0, :], qtre[:, 0, cs(1)])
            nc.sync.dma_start(kt_c[0][:, 1, :], ktre[:, 1, cs(0)])
            nc.scalar.dma_start(qt_c[1][:, 1, :], qtre[:, 1, cs(1)])
            nc.gpsimd.dma_start(vb_c[0][:], vre[:, 0:TPB])
            nc.sync.dma_start(kt_c[1][:], ktre[:, :, cs(1)])
            nc.gpsimd.dma_start(vb_c[1][:], vre[:, TPB : 2 * TPB])
            nc.sync.dma_start(qt_c[2][:], qtre[:, :, cs(2)])
            nc.sync.dma_start(kt_c[2][:], ktre[:, :, cs(2)])
            nc.gpsimd.dma_start(vb_c[2][:], vre[:, 2 * TPB : 3 * TPB])
            nc.sync.dma_start(qt_c[3][:], qtre[:, :, cs(3)])
            nc.sync.dma_start(kt_c[3][:], ktre[:, :, cs(3)])
            nc.gpsimd.dma_start(vb_c[3][:], vre[:, 3 * TPB : 4 * TPB])
            nc.sync.dma_start(qt_c[0][:], qtre[:, :, cs(0)])

            def kt_tile(h, i):
                return kt_c[i // TPB][:, h, (i % TPB) * P : (i % TPB + 1) * P]

            def v_tile(h, i):
                return vb_c[i // TPB][:, i % TPB, h, :]

            psum_o = {}
            psum_l = {}
            acc = {}  # (h, b) -> current fresh fp16 accumulator tile
            expt_of = {}
            l_alt = {}  # b0 h1's l target (ps-pool tile)

            def ensure_block(b):
                if (0, b) in psum_o:
                    return
                for h in range(HC):
                    psum_o[h, b] = po_pool.tile(
                        [P, SBLK], F32, tag="po", name=f"po{h}_{b}"
                    )
                    if b:  # b0's l lives in a spare ps tile instead
                        psum_l[h, b] = pl_pool.tile(
                            [P, SBLK], F32, tag="pl", name=f"pl{h}_{b}"
                        )

            def emit_mm1(h, b, grp):
                i0, i1, s0, s1, is_diag = grp
                psum_s = ps_pool.tile([P, 2, SBLK], F32, tag="ps", name=f"ps{h}_{b}_{i0}")
                expt = epool.tile([P, 2, SBLK], FP16, tag="expt", name=f"ex{h}_{b}_{i0}")
                for j, (i, s_lo) in enumerate(((i0, s0), (i1, s1))):
                    nc.tensor.matmul(
                        psum_s[:, j, s_lo:],
                        kt_tile(h, i),
                        qt_c[b][:, h, s_lo:],
                        start=True,
                        stop=True,
                    )
                nc.scalar.activation(
                    expt[:, :, s0:],
                    psum_s[:, :, s0:],
                    mybir.ActivationFunctionType.Exp,
                    scale=SCALE,
                )
                if is_diag:
                    # h1's masks of the last block's LAST group go to DVE so
                    # the tail's final four masks run on two engines at once
                    meng = nc.vector if (b == 0 and h == 1 and s0 != 0) else nc.gpsimd
                    for j, s_lo in enumerate((s0, s1)):
                        meng.tensor_mul(
                            out=expt[:, j, s_lo : s_lo + P],
                            in0=expt[:, j, s_lo : s_lo + P],
                            in1=tri[:],
                        )
                expt_of[h, b, i0] = expt

            def emit_diag_l(h, b, grp, target):
                """ones-matmul denominator contributions of a diag group."""
                i0, i1, s0, s1, _ = grp
                n_full = TPB * b
                last_i = n_full + TPB - 1
                expt = expt_of[h, b, i0]
                for j, (i, s_lo) in enumerate(((i0, s0), (i1, s1))):
                    nc.tensor.matmul(
                        target[:, s_lo:],
                        ones[:],
                        expt[:, j, s_lo:],
                        start=(i == n_full),
                        stop=(i == last_i and n_full == 0),
                        skip_group_check=True,
                    )

            def emit_mm2(h, b, grp, with_diag_l=True):
                i0, i1, s0, s1, is_diag = grp
                n_full = TPB * b
                last_i = n_full + TPB - 1
                expt = expt_of[h, b, i0]
                for j, (i, s_lo) in enumerate(((i0, s0), (i1, s1))):
                    nc.tensor.matmul(
                        psum_o[h, b][:, s_lo:],
                        v_tile(h, i),
                        expt[:, j, s_lo:],
                        start=(i == 0),
                        stop=(i == last_i),
                        skip_group_check=True,
                    )
                if is_diag:
                    if with_diag_l:
                        emit_diag_l(h, b, grp, psum_l[h, b])
                        expt_of.pop((h, b, i0))
                    # else: deferred — tile stays alive in expt_of
                else:
                    expt_of.pop((h, b, i0))
                    # fresh-output fp16 adds on DVE (in-place accumulate and
                    # gpsimd [P,512] adds are 2.5-3x slower)
                    pair = npool.tile([P, SBLK], FP16, tag="pair", name=f"pr{h}_{b}_{i0}")
                    nc.vector.tensor_add(
                        out=pair[:], in0=expt[:, 0, :], in1=expt[:, 1, :]
                    )
                    if (h, b) not in acc:
                        acc[h, b] = pair
                    else:
                        nacc = npool.tile(
                            [P, SBLK], FP16, tag="acc", name=f"ac{h}_{b}_{i0}"
                        )
                        nc.vector.tensor_add(
                            out=nacc[:], in0=acc[h, b][:], in1=pair[:]
                        )
                        acc[h, b] = nacc

            def emit_end(h, b, ltile):
                bs = slice(b * SBLK, (b + 1) * SBLK)
                if b:
                    # contract the DVE pair-sums over the partition dim
                    nc.tensor.matmul(
                        ltile[:],
                        ones[:],
                        acc[h, b][:],
                        start=False,
                        stop=True,
                        skip_group_check=True,
                    )
                recip = npool.tile([P, SBLK], F32, tag="recip", name=f"rc{h}_{b}")
                nc.vector.reciprocal_approx_fast(out=recip[:], in_=ltile[:])
                otn = npool.tile([P, SBLK], FP16, tag="otn", name=f"ot{h}_{b}")
                nc.vector.tensor_mul(out=otn[:], in0=psum_o[h, b][:], in1=recip[:])
                nc.sync.dma_start(ot_d[h, :, bs], otn[:])

            def emit_half_end_both(b, lps, half):
                """Normalize+store one query-half of the last block (both
                heads) as soon as causality makes it final: ONE joint recip
                over [P, 2, 256], then per-head mul + store."""
                hw2 = SBLK // 2
                hs = slice(half * hw2, (half + 1) * hw2)
                ds = slice(b * SBLK + half * hw2, b * SBLK + (half + 1) * hw2)
                recip = npool.tile([P, 2, hw2], F32, tag="reciph", name=f"rch_{b}_{half}")
                nc.vector.reciprocal_approx_fast(out=recip[:], in_=lps[:, :, hs])
                for h in range(HC):
                    otn = npool.tile([P, hw2], FP16, tag="otnh", name=f"oth{h}_{b}_{half}")
                    nc.vector.tensor_mul(
                        out=otn[:], in0=psum_o[h, b][:, hs], in1=recip[:, 1 - h, :]
                    )
                    (nc.sync if h == 0 else nc.gpsimd).dma_start(ot_d[h, :, ds], otn[:])

            # ---- flat pipeline across blocks at (head, group) granularity,
            # lookahead 2 jobs (= one group) ----
            jobs = []
            for b in BLOCK_ORDER:
                gs = _groups_of(b)
                for gi, grp in enumerate(gs):
                    for h in range(HC):
                        jobs.append((h, b, grp, gi == len(gs) - 1))
            # h0-major start: h1's gating DMAs land ~1.2us after h0's, so
            # run h0's first two groups before touching h1
            jobs[0:4] = [jobs[0], jobs[2], jobs[1], jobs[3]]

            def emit_deferred_h1(db):
                for g2 in _groups_of(db):
                    if g2[4]:
                        emit_diag_l(1, db, g2, psum_l[1, db])
                        expt_of.pop((1, db, g2[0]))
                emit_end(1, db, psum_l[1, db])

            l_ps = None  # b0's l lives in a spare ps tile (both heads)

            def consume_job(job):
                nonlocal l_ps, deferred
                h, b, grp, lastg = job
                if b != 0:
                    emit_mm2(h, b, grp, with_diag_l=(h == 0))
                    if lastg:
                        if h == 0:
                            emit_end(0, b, psum_l[0, b])
                        else:
                            # h1 (psum_l alias): defer past the next mm1s
                            deferred = b
                    return
                # b0 (the last block, diag-only): per-group diag-l into the
                # spare ps tile; queries [0:256] are final after g0 (causal),
                # so normalize+store that half while g1 still computes
                if l_ps is None:
                    l_ps = ps_pool.tile([P, 2, SBLK], F32, tag="ps", name="l_b0")
                half = 0 if grp[0] == 0 else 1
                emit_mm2(h, b, grp, with_diag_l=False)
                emit_diag_l(h, b, grp, l_ps[:, 1 - h, :])
                expt_of.pop((h, b, grp[0]))
                if h == 1:  # both heads' diag-l for this half are in
                    emit_half_end_both(b, l_ps, half)

            LOOK = 2
            deferred = None  # block whose h1 end-chain is due
            defer_age = 0
            for k, job in enumerate(jobs):
                h, b, grp, lastg = job
                ensure_block(b)
                emit_mm1(h, b, grp)
                if deferred is not None:
                    defer_age += 1
                    if defer_age >= 2:
                        db = deferred
                        deferred = None
                        defer_age = 0
                        emit_deferred_h1(db)
                if k >= LOOK:
                    consume_job(jobs[k - LOOK])
            if deferred is not None:
                emit_deferred_h1(deferred)
                deferred = None
            for k in range(len(jobs) - LOOK, len(jobs)):
                consume_job(jobs[k])
    nc.compile()
    return nc


_NC_CACHE = None


def _get_nc():
    global _NC_CACHE
    if _NC_CACHE is None:
        _NC_CACHE = build_nc()
    return _NC_CACHE


def make_in_maps(query, key, value):
    query = np.asarray(query)
    key = np.asarray(key)
    value = np.asarray(value)
    in_maps = []
    for c in range(NCORES):
        hs = slice(c * HC, (c + 1) * HC)
        in_maps.append(
            {
                "qt": np.ascontiguousarray(
                    query[0, :, hs, :].transpose(1, 2, 0)
                ).astype(np.float16),
                "kt": np.ascontiguousarray(
                    key[0, :, hs, :].transpose(1, 2, 0)
                ).astype(np.float16),
                "v": np.ascontiguousarray(value[0, :, hs, :]).astype(np.float16),
            }
        )
    return in_maps


def kernel(query, key, value):
    from concourse.bass_utils import run_bass_kernel_spmd

    nc = _get_nc()
    in_maps = make_in_maps(query, key, value)
    res = run_bass_kernel_spmd(nc, in_maps, core_ids=list(range(NCORES)))
    out = np.empty((1, S, H, D), dtype=np.float32)
    for c in range(NCORES):
        # ot is [HC, D, S] fp16 -> [S, HC, D] fp32
        out[0, :, c * HC : (c + 1) * HC, :] = (
            res.results[c]["ot"].astype(np.float32).transpose(2, 0, 1)
        )
    return out
